# revision 24
# baseline (speedup 1.0000x reference)
"""Trainium2 Bass kernel for nn_BinaryLinear (binarized 4-layer MLP + BatchNorm).

Reference computation (fp32, jax):
    h = x.reshape(-1, 3072)
    h = relu(h @ sign(W1).T); h = BN(h, g1, b1)   # BN over full 8192 batch
    h = relu(h @ sign(W2).T); h = BN(h, g2, b2)
    h = relu(h @ sign(W3).T); h = BN(h, g3, b3)
    out = h @ sign(W4).T                          # [8192, 10]

Strategy (8 NeuronCores, data-parallel over batch):
  - Host: binarize weights to bf16 (+-1 exact), pack everything partition-
    major ([128, ktiles, free]) so DMAs are fat 2D-contiguous transfers,
    shard x over cores (1024 rows each).
  - Device (SPMD identical program): activations live feature-major
    [feature_part, batch_free] in SBUF. Each layer is a K-tiled bf16 matmul
    accumulating in PSUM, ordered k-outer over groups of 4 feature tiles so
    the in-order TensorE stream consumes input tiles in DMA-arrival order
    (no head-of-line blocking on the layer-1 feed). Consecutive matmuls of
    the two batch halves share stationary weights (2nd sets ldweights=False).
    Relu on ScalarE (free per-partition sum via accum_out), sum(h^2) via a
    second ScalarE Square pass with accum_out.
  - BatchNorm over the full batch: AllGather the per-core (sum, sumsq) stats
    (one [128,16] f32 tile per layer), tree-reduce locally, apply a*h+c per
    feature via VectorE tensor_scalar. A warmup AllGather at kernel start
    absorbs the ~11us ncfw wake latency.
"""
import os
import sys

for _p in ("/opt/trn_rl_repo",):
    if os.path.isdir(_p) and _p not in sys.path:
        sys.path.insert(0, _p)

import numpy as np
import ml_dtypes

from concourse import bacc, tile, mybir
from concourse import bass_utils

NCORES = 8
B = 8192
BL = B // NCORES            # 1024 rows per core
KIN = 3072
KT_IN = KIN // 128          # 24 k-tiles for layer 1
HID = 1024
JT = HID // 128             # 8 feature tiles
CLS = 10
CLSP = 16                   # padded classes
EPS = 1e-5
BF16 = mybir.dt.bfloat16
F32 = mybir.dt.float32
ADD = mybir.AluOpType.add
SUB = mybir.AluOpType.subtract
MUL = mybir.AluOpType.mult
RELU = mybir.ActivationFunctionType.Relu
SQUARE = mybir.ActivationFunctionType.Square

_CACHE = {}


def _build(stage=99):
    nc = bacc.Bacc("TRN2", target_bir_lowering=False, debug=False, num_devices=NCORES)

    # All bulk inputs are partition-major on the host ([128, ktiles, free])
    # so DMAs are cheap-descriptor 2D patterns at full bandwidth.
    xt_d = nc.dram_tensor("xt", [128, KT_IN, BL], BF16, kind="ExternalInput")
    w1_d = nc.dram_tensor("w1t", [128, KT_IN, HID], BF16, kind="ExternalInput")
    w2_d = nc.dram_tensor("w2t", [128, JT, HID], BF16, kind="ExternalInput")
    w3_d = nc.dram_tensor("w3t", [128, JT, HID], BF16, kind="ExternalInput")
    w4_d = nc.dram_tensor("w4t", [128, JT, CLSP], BF16, kind="ExternalInput")
    bnp_d = nc.dram_tensor("bnp", [128, 6 * JT], F32, kind="ExternalInput")
    out_d = nc.dram_tensor("out", [CLSP, BL], F32, kind="ExternalOutput")

    nhalves = [(s, min(512, BL - s)) for s in range(0, BL, 512)]

    with tile.TileContext(nc) as tc:
        with (
            tc.tile_pool(name="weights", bufs=1) as wpool,
            tc.tile_pool(name="acts", bufs=1) as apool,
            tc.tile_pool(name="scratch", bufs=2) as scrpool,
            tc.tile_pool(name="stats", bufs=2) as spool,
            tc.tile_pool(name="psum", bufs=4, space="PSUM") as pspool,
            tc.tile_pool(name="dram", bufs=2, space="DRAM") as dpool,
        ):
            XT = wpool.tile([128, KT_IN, BL], BF16, tag="XT")
            W1 = wpool.tile([128, KT_IN, HID], BF16, tag="W1")
            W2 = wpool.tile([128, JT, HID], BF16, tag="W2")
            W3 = wpool.tile([128, JT, HID], BF16, tag="W3")
            W4 = wpool.tile([128, JT, CLSP], BF16, tag="W4")
            BNP = wpool.tile([128, 6 * JT], F32, tag="BNP")
            HRAW = apool.tile([128, JT, BL], BF16, tag="HRAW")
            H = apool.tile([128, JT, BL], BF16, tag="H")

            # Warmup collective: absorbs the ncfw wake latency off the
            # critical path. Input is an unwritten scratch buffer (contents
            # irrelevant); output anchored into an unused out_d row (via the
            # otherwise-idle gpsimd queue) so DCE keeps it.
            wu_in = dpool.tile([128, 1], F32, tag="wu_in")
            wu_out = dpool.tile([NCORES * 128, 1], F32, tag="wu_out")
            nc.gpsimd.collective_compute(
                "AllGather",
                mybir.AluOpType.bypass,
                replica_groups=[list(range(NCORES))],
                ins=[wu_in.opt()],
                outs=[wu_out.opt()],
            )
            nc.gpsimd.dma_start(out_d[CLSP - 1 : CLSP, 0:1], wu_out[0:1, :])

            # Input feed: W1 as one fat transfer on the GpSimd SWDGE ring;
            # XT chunked across the Sync and Scalar HWDGE rings (small first
            # chunk so the first accumulation chains start early).
            nc.sync.dma_start(BNP[:], bnp_d[:])
            nc.gpsimd.dma_start(W1[:], w1_d[:])
            chunks = [(0, 3, nc.sync), (3, 9, nc.scalar), (12, KT_IN - 12, nc.sync)]
            for c, w, eng in chunks:
                w = min(w, KT_IN - c)
                if w > 0:
                    eng.dma_start(XT[:, c : c + w, :], xt_d[:, c : c + w, :])

            def mm_pair(ps, Wk, rhs, k, kt):
                for idx, (s, w) in enumerate(nhalves):
                    mi = nc.tensor.matmul(
                        ps[:, s : s + w],
                        Wk,
                        rhs[:, k, s : s + w],
                        start=(k == 0),
                        stop=(k == kt - 1),
                    )
                    if idx > 0:
                        # same stationary weights as the previous matmul:
                        # skip the redundant LDWEIGHTS
                        mi.ins.ldweights = False

            def relu_square(ps, jt, S):
                # relu: PSUM f32 -> SBUF bf16; accum = sum over batch
                nc.scalar.activation(
                    HRAW[:, jt, :], ps[:], RELU,
                    accum_out=S[:, jt : jt + 1],
                )
                # sum of squares over batch
                scr = scrpool.tile([128, BL], BF16, tag="scr")
                nc.scalar.activation(
                    scr[:], HRAW[:, jt, :], SQUARE,
                    accum_out=S[:, JT + jt : JT + jt + 1],
                )

            def mlp_layer(kt, rhs, W, S):
                """K-tiled matmuls + relu + per-core BN stats into S.

                First (up to) 4 feature tiles run k-outer as a group, so the
                in-order TensorE stream consumes layer-1 input tiles in DMA
                arrival order. The remaining tiles run one chain at a time so
                their relu/square pipeline under the next chain's matmuls
                (small exposed tail before the BN collective)."""
                gn = min(4, JT)
                pss = [
                    pspool.tile([128, BL], F32, tag="ps", name=f"ps_g{j}")
                    for j in range(gn)
                ]
                for k in range(kt):
                    for j in range(gn):
                        mm_pair(pss[j], W[:, k, j * 128 : (j + 1) * 128], rhs, k, kt)
                for j in range(gn):
                    relu_square(pss[j], j, S)
                for jt in range(gn, JT):
                    ps = pspool.tile([128, BL], F32, tag="ps", name="ps_seq")
                    for k in range(kt):
                        mm_pair(ps, W[:, k, jt * 128 : (jt + 1) * 128], rhs, k, kt)
                    relu_square(ps, jt, S)

            def bn_sync_apply(li, S):
                """AllGather per-core stats, compute a/c, H = a*HRAW + c."""
                cc_in = dpool.tile([128, 2 * JT], F32, tag="cc_in")
                cc_out = dpool.tile([NCORES * 128, 2 * JT], F32, tag="cc_out")
                nc.sync.dma_start(cc_in[:], S[:])
                nc.gpsimd.collective_compute(
                    "AllGather",
                    mybir.AluOpType.bypass,
                    replica_groups=[list(range(NCORES))],
                    ins=[cc_in.opt()],
                    outs=[cc_out.opt()],
                )
                GAT = spool.tile([128, NCORES, 2 * JT], F32, tag="GAT")
                nc.scalar.dma_start(
                    GAT[:], cc_out.opt().rearrange("(c p) s -> p c s", p=128)
                )
                T4 = spool.tile([128, 4, 2 * JT], F32, tag="T4")
                nc.vector.tensor_tensor(T4[:], GAT[:, 0:4, :], GAT[:, 4:8, :], ADD)
                T2 = spool.tile([128, 2, 2 * JT], F32, tag="T2")
                nc.vector.tensor_tensor(T2[:], T4[:, 0:2, :], T4[:, 2:4, :], ADD)
                SS = spool.tile([128, 2 * JT], F32, tag="SS")
                nc.vector.tensor_tensor(SS[:], T2[:, 0, :], T2[:, 1, :], ADD)

                MEAN = spool.tile([128, JT], F32, tag="MEAN")
                nc.vector.tensor_scalar_mul(MEAN[:], SS[:, 0:JT], 1.0 / B)
                # E[h^2] + eps
                VPE = spool.tile([128, JT], F32, tag="VPE")
                nc.vector.tensor_scalar(
                    VPE[:], SS[:, JT : 2 * JT], 1.0 / B, EPS, MUL, ADD
                )
                MSQ = spool.tile([128, JT], F32, tag="MSQ")
                nc.vector.tensor_tensor(MSQ[:], MEAN[:], MEAN[:], MUL)
                VAR = spool.tile([128, JT], F32, tag="VAR")
                nc.vector.tensor_tensor(VAR[:], VPE[:], MSQ[:], SUB)  # var+eps
                RINV = spool.tile([128, JT], F32, tag="RINV")
                nc.vector.reciprocal(RINV[:], VAR[:])
                RSTD = spool.tile([128, JT], F32, tag="RSTD")
                nc.scalar.sqrt(RSTD[:], RINV[:])
                A = spool.tile([128, JT], F32, tag="A")
                nc.vector.tensor_tensor(
                    A[:], RSTD[:], BNP[:, (2 * li) * JT : (2 * li + 1) * JT], MUL
                )
                AM = spool.tile([128, JT], F32, tag="AM")
                nc.vector.tensor_tensor(AM[:], A[:], MEAN[:], MUL)
                C = spool.tile([128, JT], F32, tag="C")
                nc.vector.tensor_tensor(
                    C[:], BNP[:, (2 * li + 1) * JT : (2 * li + 2) * JT], AM[:], SUB
                )
                for jt in range(JT):
                    nc.vector.tensor_scalar(
                        H[:, jt, :],
                        HRAW[:, jt, :],
                        A[:, jt : jt + 1],
                        C[:, jt : jt + 1],
                        MUL,
                        ADD,
                    )

            # ---- layer 1 ----
            if stage >= 1:
                S1 = spool.tile([128, 2 * JT], F32, tag="S")
                mlp_layer(KT_IN, XT, W1, S1)
            # prefetch W2 during L1 compute
            nc.scalar.dma_start(W2[:], w2_d[:])
            if stage >= 2:
                bn_sync_apply(0, S1)

            if stage >= 3:
                # ---- layer 2 ----
                S2 = spool.tile([128, 2 * JT], F32, tag="S")
                mlp_layer(JT, H, W2, S2)
            nc.scalar.dma_start(W3[:], w3_d[:])
            if stage >= 3:
                bn_sync_apply(1, S2)

            if stage >= 4:
                # ---- layer 3 ----
                S3 = spool.tile([128, 2 * JT], F32, tag="S")
                mlp_layer(JT, H, W3, S3)
            nc.sync.dma_start(W4[:], w4_d[:])
            if stage >= 4:
                bn_sync_apply(2, S3)

            OUTS = spool.tile([CLSP, BL], F32, tag="OUTS")
            if stage >= 5:
                # ---- layer 4 (no relu/BN) ----
                ps4 = pspool.tile([CLSP, BL], F32, tag="ps")
                for k in range(JT):
                    mm_pair(ps4, W4[:, k, :], H, k, JT)
                nc.scalar.copy(OUTS[:], ps4[:])
            else:
                nc.vector.memset(OUTS[:], 0.0)
            nc.sync.dma_start(out_d[:], OUTS[:])

    nc.compile()
    return nc


def _get_nc():
    if "nc" not in _CACHE:
        _CACHE["nc"] = _build()
    return _CACHE["nc"]


def _prep_inputs(x, W1, W2, W3, W4, g1, b1, g2, b2, g3, b3):
    x2 = np.asarray(x, dtype=np.float32).reshape(B, KIN)
    xt = np.ascontiguousarray(x2.T).astype(ml_dtypes.bfloat16)  # [3072, 8192]

    def pmajor(a):
        # [ktiles*128, free] -> [128, ktiles, free] (partition-major)
        kt = a.shape[0] // 128
        return np.ascontiguousarray(
            a.reshape(kt, 128, a.shape[1]).transpose(1, 0, 2)
        )

    def bin_t(w, pad=None):
        wb = np.where(np.asarray(w, dtype=np.float32) >= 0, 1.0, -1.0)
        wt = np.ascontiguousarray(wb.T).astype(ml_dtypes.bfloat16)  # [in, out]
        if pad is not None and wt.shape[1] < pad:
            wt = np.concatenate(
                [wt, np.zeros((wt.shape[0], pad - wt.shape[1]), wt.dtype)], axis=1
            )
        return pmajor(wt)

    w1t = bin_t(W1)            # [128, 24, 1024]
    w2t = bin_t(W2)            # [128, 8, 1024]
    w3t = bin_t(W3)
    w4t = bin_t(W4, pad=CLSP)  # [128, 8, 16]

    bnp = np.zeros((128, 6 * JT), dtype=np.float32)
    for l, p in enumerate([g1, b1, g2, b2, g3, b3]):
        pa = np.asarray(p, dtype=np.float32)
        for jt in range(JT):
            bnp[:, l * JT + jt] = pa[jt * 128 : (jt + 1) * 128]

    shared = {"w1t": w1t, "w2t": w2t, "w3t": w3t, "w4t": w4t, "bnp": bnp}
    in_maps = []
    for c in range(NCORES):
        m = dict(shared)
        m["xt"] = pmajor(np.ascontiguousarray(xt[:, c * BL : (c + 1) * BL]))
        in_maps.append(m)
    return in_maps


def _run(inputs, trace=False):
    nc = _get_nc()
    in_maps = _prep_inputs(**inputs)
    res = bass_utils.run_bass_kernel_spmd(
        nc, in_maps, core_ids=list(range(NCORES)), trace=trace
    )
    out = np.empty((B, CLS), dtype=np.float32)
    for c in range(NCORES):
        out[c * BL : (c + 1) * BL, :] = res.results[c]["out"][:CLS, :].T
    return out, res


def kernel(**inputs):
    out, _ = _run(inputs, trace=False)
    return out


# revision 26
# speedup vs baseline: 1.0940x; 1.0940x over previous
"""Trainium2 Bass kernel for nn_BinaryLinear (binarized 4-layer MLP + BatchNorm).

Reference computation (fp32, jax):
    h = x.reshape(-1, 3072)
    h = relu(h @ sign(W1).T); h = BN(h, g1, b1)   # BN over full 8192 batch
    h = relu(h @ sign(W2).T); h = BN(h, g2, b2)
    h = relu(h @ sign(W3).T); h = BN(h, g3, b3)
    out = h @ sign(W4).T                          # [8192, 10]

Strategy (8 NeuronCores, data-parallel over batch):
  - Host: binarize weights to bf16 (+-1 exact), pack everything partition-
    major ([128, ktiles, free]) so DMAs are fat 2D-contiguous transfers,
    shard x over cores (1024 rows each).
  - Device (SPMD identical program): activations live feature-major
    [feature_part, batch_free] in SBUF. Each layer is a K-tiled bf16 matmul
    accumulating in PSUM, ordered k-outer over groups of 4 feature tiles so
    the in-order TensorE stream consumes input tiles in DMA-arrival order
    (no head-of-line blocking on the layer-1 feed). Consecutive matmuls of
    the two batch halves share stationary weights (2nd sets ldweights=False).
    Relu on ScalarE (free per-partition sum via accum_out), sum(h^2) via a
    second ScalarE Square pass with accum_out.
  - BatchNorm over the full batch: AllGather the per-core (sum, sumsq) stats
    (one [128,16] f32 tile per layer), tree-reduce locally, apply a*h+c per
    feature via VectorE tensor_scalar. A warmup AllGather at kernel start
    absorbs the ~11us ncfw wake latency.
"""
import os
import sys

for _p in ("/opt/trn_rl_repo",):
    if os.path.isdir(_p) and _p not in sys.path:
        sys.path.insert(0, _p)

import numpy as np
import ml_dtypes

from concourse import bacc, tile, mybir
from concourse import bass_utils

NCORES = 8
B = 8192
BL = B // NCORES            # 1024 rows per core
KIN = 3072
KT_IN = KIN // 128          # 24 k-tiles for layer 1
HID = 1024
JT = HID // 128             # 8 feature tiles
CLS = 10
CLSP = 16                   # padded classes
EPS = 1e-5
BF16 = mybir.dt.bfloat16
F32 = mybir.dt.float32
ADD = mybir.AluOpType.add
SUB = mybir.AluOpType.subtract
MUL = mybir.AluOpType.mult
RELU = mybir.ActivationFunctionType.Relu
SQUARE = mybir.ActivationFunctionType.Square

_CACHE = {}


def _build(stage=99):
    nc = bacc.Bacc("TRN2", target_bir_lowering=False, debug=False, num_devices=NCORES)

    # All bulk inputs are partition-major on the host ([128, ktiles, free])
    # so DMAs are cheap-descriptor 2D patterns at full bandwidth.
    xt_d = nc.dram_tensor("xt", [128, KT_IN, BL], BF16, kind="ExternalInput")
    w1_d = nc.dram_tensor("w1t", [128, KT_IN, HID], BF16, kind="ExternalInput")
    w2_d = nc.dram_tensor("w2t", [128, JT, HID], BF16, kind="ExternalInput")
    w3_d = nc.dram_tensor("w3t", [128, JT, HID], BF16, kind="ExternalInput")
    w4_d = nc.dram_tensor("w4t", [128, JT, CLSP], BF16, kind="ExternalInput")
    bnp_d = nc.dram_tensor("bnp", [128, 6 * JT], F32, kind="ExternalInput")
    out_d = nc.dram_tensor("out", [CLSP, BL], F32, kind="ExternalOutput")

    nhalves = [(s, min(512, BL - s)) for s in range(0, BL, 512)]

    with tile.TileContext(nc) as tc:
        with (
            tc.tile_pool(name="weights", bufs=1) as wpool,
            tc.tile_pool(name="acts", bufs=1) as apool,
            tc.tile_pool(name="scratch", bufs=2) as scrpool,
            tc.tile_pool(name="stats", bufs=2) as spool,
            tc.tile_pool(name="psum", bufs=4, space="PSUM") as pspool,
            tc.tile_pool(name="dram", bufs=2, space="DRAM") as dpool,
        ):
            XT = wpool.tile([128, KT_IN, BL], BF16, tag="XT")
            W1 = wpool.tile([128, KT_IN, HID], BF16, tag="W1")
            W2 = wpool.tile([128, JT, HID], BF16, tag="W2")
            W3 = wpool.tile([128, JT, HID], BF16, tag="W3")
            W4 = wpool.tile([128, JT, CLSP], BF16, tag="W4")
            BNP = wpool.tile([128, 6 * JT], F32, tag="BNP")
            HRAW = apool.tile([128, JT, BL], BF16, tag="HRAW")
            H = apool.tile([128, JT, BL], BF16, tag="H")

            # Warmup collective: absorbs the ncfw wake latency off the
            # critical path. Input is an unwritten scratch buffer (contents
            # irrelevant); output anchored into an unused out_d row (via the
            # otherwise-idle gpsimd queue) so DCE keeps it.
            wu_in = dpool.tile([128, 1], F32, tag="wu_in")
            wu_out = dpool.tile([NCORES * 128, 1], F32, tag="wu_out")
            nc.gpsimd.collective_compute(
                "AllGather",
                mybir.AluOpType.bypass,
                replica_groups=[list(range(NCORES))],
                ins=[wu_in.opt()],
                outs=[wu_out.opt()],
            )
            nc.gpsimd.dma_start(out_d[CLSP - 1 : CLSP, 0:1], wu_out[0:1, :])

            # Input feed: XT on the Sync HWDGE ring, W1 on the Scalar HWDGE
            # ring, in progressively larger chunks so the first accumulation
            # chains start early while the bulk still moves in fat transfers.
            nc.sync.dma_start(BNP[:], bnp_d[:])
            for c, w in ((0, 4), (4, 8), (12, KT_IN - 12)):
                w = min(w, KT_IN - c)
                if w > 0:
                    nc.sync.dma_start(XT[:, c : c + w, :], xt_d[:, c : c + w, :])
                    nc.scalar.dma_start(W1[:, c : c + w, :], w1_d[:, c : c + w, :])

            def mm_pair(ps, Wk, rhs, k, kt):
                for idx, (s, w) in enumerate(nhalves):
                    mi = nc.tensor.matmul(
                        ps[:, s : s + w],
                        Wk,
                        rhs[:, k, s : s + w],
                        start=(k == 0),
                        stop=(k == kt - 1),
                    )
                    if idx > 0:
                        # same stationary weights as the previous matmul:
                        # skip the redundant LDWEIGHTS
                        mi.ins.ldweights = False

            def relu_square(ps, jt, S):
                # relu: PSUM f32 -> SBUF bf16; accum = sum over batch
                nc.scalar.activation(
                    HRAW[:, jt, :], ps[:], RELU,
                    accum_out=S[:, jt : jt + 1],
                )
                # sum of squares over batch
                scr = scrpool.tile([128, BL], BF16, tag="scr")
                nc.scalar.activation(
                    scr[:], HRAW[:, jt, :], SQUARE,
                    accum_out=S[:, JT + jt : JT + jt + 1],
                )

            def mlp_layer(kt, rhs, W, S):
                """K-tiled matmuls + relu + per-core BN stats into S.

                First (up to) 4 feature tiles run k-outer as a group, so the
                in-order TensorE stream consumes layer-1 input tiles in DMA
                arrival order. The remaining tiles run one chain at a time so
                their relu/square pipeline under the next chain's matmuls
                (small exposed tail before the BN collective)."""
                gn = min(4, JT)
                pss = [
                    pspool.tile([128, BL], F32, tag="ps", name=f"ps_g{j}")
                    for j in range(gn)
                ]
                for k in range(kt):
                    for j in range(gn):
                        mm_pair(pss[j], W[:, k, j * 128 : (j + 1) * 128], rhs, k, kt)
                for j in range(gn):
                    relu_square(pss[j], j, S)
                for jt in range(gn, JT):
                    ps = pspool.tile([128, BL], F32, tag="ps", name="ps_seq")
                    for k in range(kt):
                        mm_pair(ps, W[:, k, jt * 128 : (jt + 1) * 128], rhs, k, kt)
                    relu_square(ps, jt, S)

            def bn_sync_apply(li, S):
                """AllGather per-core stats, compute a/c, H = a*HRAW + c."""
                cc_in = dpool.tile([128, 2 * JT], F32, tag="cc_in")
                cc_out = dpool.tile([NCORES * 128, 2 * JT], F32, tag="cc_out")
                nc.sync.dma_start(cc_in[:], S[:])
                nc.gpsimd.collective_compute(
                    "AllGather",
                    mybir.AluOpType.bypass,
                    replica_groups=[list(range(NCORES))],
                    ins=[cc_in.opt()],
                    outs=[cc_out.opt()],
                )
                GAT = spool.tile([128, NCORES, 2 * JT], F32, tag="GAT")
                nc.scalar.dma_start(
                    GAT[:], cc_out.opt().rearrange("(c p) s -> p c s", p=128)
                )
                T4 = spool.tile([128, 4, 2 * JT], F32, tag="T4")
                nc.vector.tensor_tensor(T4[:], GAT[:, 0:4, :], GAT[:, 4:8, :], ADD)
                T2 = spool.tile([128, 2, 2 * JT], F32, tag="T2")
                nc.vector.tensor_tensor(T2[:], T4[:, 0:2, :], T4[:, 2:4, :], ADD)
                SS = spool.tile([128, 2 * JT], F32, tag="SS")
                nc.vector.tensor_tensor(SS[:], T2[:, 0, :], T2[:, 1, :], ADD)

                MEAN = spool.tile([128, JT], F32, tag="MEAN")
                nc.vector.tensor_scalar_mul(MEAN[:], SS[:, 0:JT], 1.0 / B)
                # E[h^2] + eps
                VPE = spool.tile([128, JT], F32, tag="VPE")
                nc.vector.tensor_scalar(
                    VPE[:], SS[:, JT : 2 * JT], 1.0 / B, EPS, MUL, ADD
                )
                MSQ = spool.tile([128, JT], F32, tag="MSQ")
                nc.vector.tensor_tensor(MSQ[:], MEAN[:], MEAN[:], MUL)
                VAR = spool.tile([128, JT], F32, tag="VAR")
                nc.vector.tensor_tensor(VAR[:], VPE[:], MSQ[:], SUB)  # var+eps
                RINV = spool.tile([128, JT], F32, tag="RINV")
                nc.vector.reciprocal(RINV[:], VAR[:])
                RSTD = spool.tile([128, JT], F32, tag="RSTD")
                nc.scalar.sqrt(RSTD[:], RINV[:])
                A = spool.tile([128, JT], F32, tag="A")
                nc.vector.tensor_tensor(
                    A[:], RSTD[:], BNP[:, (2 * li) * JT : (2 * li + 1) * JT], MUL
                )
                AM = spool.tile([128, JT], F32, tag="AM")
                nc.vector.tensor_tensor(AM[:], A[:], MEAN[:], MUL)
                C = spool.tile([128, JT], F32, tag="C")
                nc.vector.tensor_tensor(
                    C[:], BNP[:, (2 * li + 1) * JT : (2 * li + 2) * JT], AM[:], SUB
                )
                for jt in range(JT):
                    nc.vector.tensor_scalar(
                        H[:, jt, :],
                        HRAW[:, jt, :],
                        A[:, jt : jt + 1],
                        C[:, jt : jt + 1],
                        MUL,
                        ADD,
                    )

            # ---- layer 1 ----
            if stage >= 1:
                S1 = spool.tile([128, 2 * JT], F32, tag="S")
                mlp_layer(KT_IN, XT, W1, S1)
            # prefetch W2 during L1 compute
            nc.sync.dma_start(W2[:], w2_d[:])
            if stage >= 2:
                bn_sync_apply(0, S1)

            if stage >= 3:
                # ---- layer 2 ----
                S2 = spool.tile([128, 2 * JT], F32, tag="S")
                mlp_layer(JT, H, W2, S2)
            nc.scalar.dma_start(W3[:], w3_d[:])
            if stage >= 3:
                bn_sync_apply(1, S2)

            if stage >= 4:
                # ---- layer 3 ----
                S3 = spool.tile([128, 2 * JT], F32, tag="S")
                mlp_layer(JT, H, W3, S3)
            nc.sync.dma_start(W4[:], w4_d[:])
            if stage >= 4:
                bn_sync_apply(2, S3)

            OUTS = spool.tile([CLSP, BL], F32, tag="OUTS")
            if stage >= 5:
                # ---- layer 4 (no relu/BN) ----
                ps4 = pspool.tile([CLSP, BL], F32, tag="ps")
                for k in range(JT):
                    mm_pair(ps4, W4[:, k, :], H, k, JT)
                nc.scalar.copy(OUTS[:], ps4[:])
            else:
                nc.vector.memset(OUTS[:], 0.0)
            nc.sync.dma_start(out_d[:], OUTS[:])

    nc.compile()
    return nc


def _get_nc():
    if "nc" not in _CACHE:
        _CACHE["nc"] = _build()
    return _CACHE["nc"]


def _prep_inputs(x, W1, W2, W3, W4, g1, b1, g2, b2, g3, b3):
    x2 = np.asarray(x, dtype=np.float32).reshape(B, KIN)
    xt = np.ascontiguousarray(x2.T).astype(ml_dtypes.bfloat16)  # [3072, 8192]

    def pmajor(a):
        # [ktiles*128, free] -> [128, ktiles, free] (partition-major)
        kt = a.shape[0] // 128
        return np.ascontiguousarray(
            a.reshape(kt, 128, a.shape[1]).transpose(1, 0, 2)
        )

    def bin_t(w, pad=None):
        wb = np.where(np.asarray(w, dtype=np.float32) >= 0, 1.0, -1.0)
        wt = np.ascontiguousarray(wb.T).astype(ml_dtypes.bfloat16)  # [in, out]
        if pad is not None and wt.shape[1] < pad:
            wt = np.concatenate(
                [wt, np.zeros((wt.shape[0], pad - wt.shape[1]), wt.dtype)], axis=1
            )
        return pmajor(wt)

    w1t = bin_t(W1)            # [128, 24, 1024]
    w2t = bin_t(W2)            # [128, 8, 1024]
    w3t = bin_t(W3)
    w4t = bin_t(W4, pad=CLSP)  # [128, 8, 16]

    bnp = np.zeros((128, 6 * JT), dtype=np.float32)
    for l, p in enumerate([g1, b1, g2, b2, g3, b3]):
        pa = np.asarray(p, dtype=np.float32)
        for jt in range(JT):
            bnp[:, l * JT + jt] = pa[jt * 128 : (jt + 1) * 128]

    shared = {"w1t": w1t, "w2t": w2t, "w3t": w3t, "w4t": w4t, "bnp": bnp}
    in_maps = []
    for c in range(NCORES):
        m = dict(shared)
        m["xt"] = pmajor(np.ascontiguousarray(xt[:, c * BL : (c + 1) * BL]))
        in_maps.append(m)
    return in_maps


def _run(inputs, trace=False):
    nc = _get_nc()
    in_maps = _prep_inputs(**inputs)
    res = bass_utils.run_bass_kernel_spmd(
        nc, in_maps, core_ids=list(range(NCORES)), trace=trace
    )
    out = np.empty((B, CLS), dtype=np.float32)
    for c in range(NCORES):
        out[c * BL : (c + 1) * BL, :] = res.results[c]["out"][:CLS, :].T
    return out, res


def kernel(**inputs):
    out, _ = _run(inputs, trace=False)
    return out


# revision 28
# speedup vs baseline: 1.1317x; 1.0344x over previous
"""Trainium2 Bass kernel for nn_BinaryLinear (binarized 4-layer MLP + BatchNorm).

Reference computation (fp32, jax):
    h = x.reshape(-1, 3072)
    h = relu(h @ sign(W1).T); h = BN(h, g1, b1)   # BN over full 8192 batch
    h = relu(h @ sign(W2).T); h = BN(h, g2, b2)
    h = relu(h @ sign(W3).T); h = BN(h, g3, b3)
    out = h @ sign(W4).T                          # [8192, 10]

Strategy (8 NeuronCores, data-parallel over batch):
  - Host: binarize weights to bf16 (+-1 exact), pack everything partition-
    major ([128, ktiles, free]) so DMAs are fat 2D-contiguous transfers,
    shard x over cores (1024 rows each).
  - Device (SPMD identical program): activations live feature-major
    [feature_part, batch_free] in SBUF. Each layer is a K-tiled bf16 matmul
    accumulating in PSUM, ordered k-outer over groups of 4 feature tiles so
    the in-order TensorE stream consumes input tiles in DMA-arrival order
    (no head-of-line blocking on the layer-1 feed). Consecutive matmuls of
    the two batch halves share stationary weights (2nd sets ldweights=False).
    Relu on ScalarE (free per-partition sum via accum_out), sum(h^2) via a
    second ScalarE Square pass with accum_out.
  - BatchNorm over the full batch: AllGather the per-core (sum, sumsq) stats
    (one [128,16] f32 tile per layer), tree-reduce locally, apply a*h+c per
    feature via VectorE tensor_scalar. A warmup AllGather at kernel start
    absorbs the ~11us ncfw wake latency.
"""
import os
import sys

for _p in ("/opt/trn_rl_repo",):
    if os.path.isdir(_p) and _p not in sys.path:
        sys.path.insert(0, _p)

import numpy as np
import ml_dtypes

from concourse import bacc, tile, mybir
from concourse import bass_utils

NCORES = 8
B = 8192
BL = B // NCORES            # 1024 rows per core
KIN = 3072
KT_IN = KIN // 128          # 24 k-tiles for layer 1
HID = 1024
JT = HID // 128             # 8 feature tiles
CLS = 10
CLSP = 16                   # padded classes
EPS = 1e-5
BF16 = mybir.dt.bfloat16
F32 = mybir.dt.float32
ADD = mybir.AluOpType.add
SUB = mybir.AluOpType.subtract
MUL = mybir.AluOpType.mult
RELU = mybir.ActivationFunctionType.Relu
SQUARE = mybir.ActivationFunctionType.Square

_CACHE = {}


def _build(stage=99):
    nc = bacc.Bacc("TRN2", target_bir_lowering=False, debug=False, num_devices=NCORES)

    # All bulk inputs are partition-major on the host ([128, ktiles, free])
    # so DMAs are cheap-descriptor 2D patterns at full bandwidth.
    xt_d = nc.dram_tensor("xt", [128, KT_IN, BL], BF16, kind="ExternalInput")
    w1_d = nc.dram_tensor("w1t", [128, KT_IN, HID], BF16, kind="ExternalInput")
    w2_d = nc.dram_tensor("w2t", [128, JT, HID], BF16, kind="ExternalInput")
    w3_d = nc.dram_tensor("w3t", [128, JT, HID], BF16, kind="ExternalInput")
    w4_d = nc.dram_tensor("w4t", [128, JT, CLSP], BF16, kind="ExternalInput")
    bnp_d = nc.dram_tensor("bnp", [128, 6 * JT], F32, kind="ExternalInput")
    out_d = nc.dram_tensor("out", [CLSP, BL], F32, kind="ExternalOutput")

    nhalves = [(s, min(512, BL - s)) for s in range(0, BL, 512)]

    with tile.TileContext(nc) as tc:
        with (
            tc.tile_pool(name="weights", bufs=1) as wpool,
            tc.tile_pool(name="acts", bufs=1) as apool,
            tc.tile_pool(name="scratch", bufs=2) as scrpool,
            tc.tile_pool(name="stats", bufs=2) as spool,
            tc.tile_pool(name="psum", bufs=4, space="PSUM") as pspool,
            tc.tile_pool(name="dram", bufs=2, space="DRAM") as dpool,
        ):
            XT = wpool.tile([128, KT_IN, BL], BF16, tag="XT")
            W1 = wpool.tile([128, KT_IN, HID], BF16, tag="W1")
            W2 = wpool.tile([128, JT, HID], BF16, tag="W2")
            W3 = wpool.tile([128, JT, HID], BF16, tag="W3")
            W4 = wpool.tile([128, JT, CLSP], BF16, tag="W4")
            BNP = wpool.tile([128, 6 * JT], F32, tag="BNP")
            HRAW = apool.tile([128, JT, BL], BF16, tag="HRAW")
            H = apool.tile([128, JT, BL], BF16, tag="H")

            # Warmup collective: absorbs the ncfw wake latency off the
            # critical path. Input is an unwritten scratch buffer (contents
            # irrelevant); output anchored into an unused out_d row (via the
            # otherwise-idle gpsimd queue) so DCE keeps it.
            wu_in = dpool.tile([128, 1], F32, tag="wu_in")
            wu_out = dpool.tile([NCORES * 128, 1], F32, tag="wu_out")
            nc.gpsimd.collective_compute(
                "AllGather",
                mybir.AluOpType.bypass,
                replica_groups=[list(range(NCORES))],
                ins=[wu_in.opt()],
                outs=[wu_out.opt()],
            )
            nc.gpsimd.dma_start(out_d[CLSP - 1 : CLSP, 0:1], wu_out[0:1, :])

            # Input feed: XT on the Sync HWDGE ring, W1 on the Scalar HWDGE
            # ring, in progressively larger chunks so the first accumulation
            # chains start early while the bulk still moves in fat transfers.
            nc.sync.dma_start(BNP[:], bnp_d[:])
            for c in range(0, KT_IN, 4):
                w = min(4, KT_IN - c)
                nc.sync.dma_start(XT[:, c : c + w, :], xt_d[:, c : c + w, :])
                nc.scalar.dma_start(W1[:, c : c + w, :], w1_d[:, c : c + w, :])

            def mm_pair(ps, Wk, rhs, k, kt):
                for idx, (s, w) in enumerate(nhalves):
                    mi = nc.tensor.matmul(
                        ps[:, s : s + w],
                        Wk,
                        rhs[:, k, s : s + w],
                        start=(k == 0),
                        stop=(k == kt - 1),
                    )
                    if idx > 0:
                        # same stationary weights as the previous matmul:
                        # skip the redundant LDWEIGHTS
                        mi.ins.ldweights = False

            def relu_square(ps, jt, S):
                # relu: PSUM f32 -> SBUF bf16; accum = sum over batch
                nc.scalar.activation(
                    HRAW[:, jt, :], ps[:], RELU,
                    accum_out=S[:, jt : jt + 1],
                )
                # sum of squares over batch (VectorE: (h bypass 0) * h, accum)
                scr = scrpool.tile([128, BL], BF16, tag="scr")
                nc.vector.scalar_tensor_tensor(
                    scr[:], HRAW[:, jt, :], 0.0, HRAW[:, jt, :],
                    mybir.AluOpType.bypass, MUL,
                    accum_out=S[:, JT + jt : JT + jt + 1],
                )

            def mlp_layer(kt, rhs, W, S):
                """K-tiled matmuls + relu + per-core BN stats into S.

                First (up to) 4 feature tiles run k-outer as a group, so the
                in-order TensorE stream consumes layer-1 input tiles in DMA
                arrival order. The remaining tiles run one chain at a time so
                their relu/square pipeline under the next chain's matmuls
                (small exposed tail before the BN collective)."""
                gn = min(4, JT)
                pss = [
                    pspool.tile([128, BL], F32, tag="ps", name=f"ps_g{j}")
                    for j in range(gn)
                ]
                for k in range(kt):
                    for j in range(gn):
                        mm_pair(pss[j], W[:, k, j * 128 : (j + 1) * 128], rhs, k, kt)
                for j in range(gn):
                    relu_square(pss[j], j, S)
                for jt in range(gn, JT):
                    ps = pspool.tile([128, BL], F32, tag="ps", name="ps_seq")
                    for k in range(kt):
                        mm_pair(ps, W[:, k, jt * 128 : (jt + 1) * 128], rhs, k, kt)
                    relu_square(ps, jt, S)

            def bn_sync_apply(li, S):
                """AllGather per-core stats, compute a/c, H = a*HRAW + c."""
                cc_in = dpool.tile([128, 2 * JT], F32, tag="cc_in")
                cc_out = dpool.tile([NCORES * 128, 2 * JT], F32, tag="cc_out")
                nc.sync.dma_start(cc_in[:], S[:])
                nc.gpsimd.collective_compute(
                    "AllGather",
                    mybir.AluOpType.bypass,
                    replica_groups=[list(range(NCORES))],
                    ins=[cc_in.opt()],
                    outs=[cc_out.opt()],
                )
                GAT = spool.tile([128, NCORES, 2 * JT], F32, tag="GAT")
                nc.scalar.dma_start(
                    GAT[:], cc_out.opt().rearrange("(c p) s -> p c s", p=128)
                )
                T4 = spool.tile([128, 4, 2 * JT], F32, tag="T4")
                nc.vector.tensor_tensor(T4[:], GAT[:, 0:4, :], GAT[:, 4:8, :], ADD)
                T2 = spool.tile([128, 2, 2 * JT], F32, tag="T2")
                nc.vector.tensor_tensor(T2[:], T4[:, 0:2, :], T4[:, 2:4, :], ADD)
                SS = spool.tile([128, 2 * JT], F32, tag="SS")
                nc.vector.tensor_tensor(SS[:], T2[:, 0, :], T2[:, 1, :], ADD)

                MEAN = spool.tile([128, JT], F32, tag="MEAN")
                nc.vector.tensor_scalar_mul(MEAN[:], SS[:, 0:JT], 1.0 / B)
                # E[h^2] + eps
                VPE = spool.tile([128, JT], F32, tag="VPE")
                nc.vector.tensor_scalar(
                    VPE[:], SS[:, JT : 2 * JT], 1.0 / B, EPS, MUL, ADD
                )
                MSQ = spool.tile([128, JT], F32, tag="MSQ")
                nc.vector.tensor_tensor(MSQ[:], MEAN[:], MEAN[:], MUL)
                VAR = spool.tile([128, JT], F32, tag="VAR")
                nc.vector.tensor_tensor(VAR[:], VPE[:], MSQ[:], SUB)  # var+eps
                RINV = spool.tile([128, JT], F32, tag="RINV")
                nc.vector.reciprocal(RINV[:], VAR[:])
                RSTD = spool.tile([128, JT], F32, tag="RSTD")
                nc.scalar.sqrt(RSTD[:], RINV[:])
                A = spool.tile([128, JT], F32, tag="A")
                nc.vector.tensor_tensor(
                    A[:], RSTD[:], BNP[:, (2 * li) * JT : (2 * li + 1) * JT], MUL
                )
                AM = spool.tile([128, JT], F32, tag="AM")
                nc.vector.tensor_tensor(AM[:], A[:], MEAN[:], MUL)
                C = spool.tile([128, JT], F32, tag="C")
                nc.vector.tensor_tensor(
                    C[:], BNP[:, (2 * li + 1) * JT : (2 * li + 2) * JT], AM[:], SUB
                )
                for jt in range(JT):
                    nc.vector.tensor_scalar(
                        H[:, jt, :],
                        HRAW[:, jt, :],
                        A[:, jt : jt + 1],
                        C[:, jt : jt + 1],
                        MUL,
                        ADD,
                    )

            # ---- layer 1 ----
            if stage >= 1:
                S1 = spool.tile([128, 2 * JT], F32, tag="S")
                mlp_layer(KT_IN, XT, W1, S1)
            # prefetch W2 during L1 compute
            nc.sync.dma_start(W2[:], w2_d[:])
            if stage >= 2:
                bn_sync_apply(0, S1)

            if stage >= 3:
                # ---- layer 2 ----
                S2 = spool.tile([128, 2 * JT], F32, tag="S")
                mlp_layer(JT, H, W2, S2)
            nc.scalar.dma_start(W3[:], w3_d[:])
            if stage >= 3:
                bn_sync_apply(1, S2)

            if stage >= 4:
                # ---- layer 3 ----
                S3 = spool.tile([128, 2 * JT], F32, tag="S")
                mlp_layer(JT, H, W3, S3)
            nc.sync.dma_start(W4[:], w4_d[:])
            if stage >= 4:
                bn_sync_apply(2, S3)

            OUTS = spool.tile([CLSP, BL], F32, tag="OUTS")
            if stage >= 5:
                # ---- layer 4 (no relu/BN) ----
                ps4 = pspool.tile([CLSP, BL], F32, tag="ps")
                for k in range(JT):
                    mm_pair(ps4, W4[:, k, :], H, k, JT)
                nc.scalar.copy(OUTS[:], ps4[:])
            else:
                nc.vector.memset(OUTS[:], 0.0)
            nc.sync.dma_start(out_d[:], OUTS[:])

    nc.compile()
    return nc


def _get_nc():
    if "nc" not in _CACHE:
        _CACHE["nc"] = _build()
    return _CACHE["nc"]


def _prep_inputs(x, W1, W2, W3, W4, g1, b1, g2, b2, g3, b3):
    x2 = np.asarray(x, dtype=np.float32).reshape(B, KIN)
    xt = np.ascontiguousarray(x2.T).astype(ml_dtypes.bfloat16)  # [3072, 8192]

    def pmajor(a):
        # [ktiles*128, free] -> [128, ktiles, free] (partition-major)
        kt = a.shape[0] // 128
        return np.ascontiguousarray(
            a.reshape(kt, 128, a.shape[1]).transpose(1, 0, 2)
        )

    def bin_t(w, pad=None):
        wb = np.where(np.asarray(w, dtype=np.float32) >= 0, 1.0, -1.0)
        wt = np.ascontiguousarray(wb.T).astype(ml_dtypes.bfloat16)  # [in, out]
        if pad is not None and wt.shape[1] < pad:
            wt = np.concatenate(
                [wt, np.zeros((wt.shape[0], pad - wt.shape[1]), wt.dtype)], axis=1
            )
        return pmajor(wt)

    w1t = bin_t(W1)            # [128, 24, 1024]
    w2t = bin_t(W2)            # [128, 8, 1024]
    w3t = bin_t(W3)
    w4t = bin_t(W4, pad=CLSP)  # [128, 8, 16]

    bnp = np.zeros((128, 6 * JT), dtype=np.float32)
    for l, p in enumerate([g1, b1, g2, b2, g3, b3]):
        pa = np.asarray(p, dtype=np.float32)
        for jt in range(JT):
            bnp[:, l * JT + jt] = pa[jt * 128 : (jt + 1) * 128]

    shared = {"w1t": w1t, "w2t": w2t, "w3t": w3t, "w4t": w4t, "bnp": bnp}
    in_maps = []
    for c in range(NCORES):
        m = dict(shared)
        m["xt"] = pmajor(np.ascontiguousarray(xt[:, c * BL : (c + 1) * BL]))
        in_maps.append(m)
    return in_maps


def _run(inputs, trace=False):
    nc = _get_nc()
    in_maps = _prep_inputs(**inputs)
    res = bass_utils.run_bass_kernel_spmd(
        nc, in_maps, core_ids=list(range(NCORES)), trace=trace
    )
    out = np.empty((B, CLS), dtype=np.float32)
    for c in range(NCORES):
        out[c * BL : (c + 1) * BL, :] = res.results[c]["out"][:CLS, :].T
    return out, res


def kernel(**inputs):
    out, _ = _run(inputs, trace=False)
    return out
